# revision 2
# baseline (speedup 1.0000x reference)
"""Trainium2 Bass kernel for nn_EnhancedSpikingRetrievalCore (sparse MoE).

The reference computes all E=8 dense expert MLPs and then combines with
renormalized top-2 gate weights -- so 6/8 of the expert FLOPs multiply
into zero gate weights.  This kernel computes only the routed
(row, expert) pairs:

  host:   bit-exact gating (same jnp ops / platform as the reference)
          -> per-expert row lists -> deal each expert's rows round-robin
          across the 8 cores (per-core per-expert counts differ by <=1,
          so one SPMD program with uniform segment capacities serves all
          cores).  Columns of X are gathered host-side; the final
          combine (sum of each row's two expert partials) is a host
          sort-gather-add, i.e. pure unshard/gather work.
  device: per active expert e (capacity C_e columns per core):
            h_e   = gw_e * relu(Xg^T[e-cols] @ W1[e] + b1[e])   [P x C_e]
            Yg^T  = W2[e]^T @ h_e                               [D x C_e]
          Both matmuls keep batch in the moving (free) dimension so
          segment sizes can be exact; weights use native layouts as the
          stationary operand.

With ~2*B/8 + padding ~ 2060 columns per core instead of 8192, the PE
does ~4x less work than the dense kernel.  Expert partials are written
in bf16 (adds ~2e-4 rms on top of the ~2.7e-3 bf16 matmul error).
"""

import numpy as np
import ml_dtypes

B, D, E, G, P, H = 8192, 2048, 8, 4, 512, 192
T, DT_LIF, TAU, VTH, VRESET = 20, 0.001, 0.02, 0.5, 0.0
DELTA0, KTOP, KROUTE, PREDW = 7.0, 32, 2, 0.1

N_CORES = 8
KC = D // 128              # contraction chunks over D (16)
PC = P // 128              # chunks over P (4)
DT_TILES = D // 128        # output D row tiles (16)
CHUNK = 512                # moving-dim chunk (one PSUM bank of fp32)

BF16 = ml_dtypes.bfloat16


def _gate_weights(x, Wg, bg, Wp, bp, Wgg, bgg):
    """Renormalized top-2 gate weights [B, E], mirroring the reference
    gating ops verbatim (same jnp calls, default platform) so the
    ill-conditioned expert selection matches the oracle bit-for-bit."""
    import jax
    import jax.numpy as jnp

    x = jnp.asarray(x)
    dtype = x.dtype
    qm = jnp.mean(x, axis=-1)
    freqs = DELTA0 * jnp.arange(1, H + 1, dtype=dtype)
    ang = qm[:, None] * freqs[None, :]
    temporal = jnp.concatenate([jnp.cos(ang), jnp.sin(ang)], axis=-1)

    # mean over D of the spike rates is exactly KTOP/D for every row
    att_mean = jnp.full((x.shape[0],), np.float32(KTOP) / np.float32(D), dtype)
    gate_in = jnp.stack([jnp.mean(temporal, axis=-1), att_mean], axis=-1)

    gate_logits = gate_in @ jnp.asarray(Wg) + jnp.asarray(bg)
    gate_logits = gate_logits - PREDW * (gate_in @ jnp.asarray(Wp) + jnp.asarray(bp))
    group_logits = gate_in @ jnp.asarray(Wgg) + jnp.asarray(bgg)
    gmap = jax.nn.one_hot(jnp.arange(E) % G, G, dtype=dtype)
    gate_logits = gate_logits + group_logits @ gmap.T

    gate_weights = jax.nn.softmax(gate_logits, axis=-1)
    _, tidx = jax.lax.top_k(gate_weights, KROUTE)
    rows = jnp.arange(x.shape[0])[:, None]
    mask = jnp.zeros_like(gate_weights).at[rows, tidx].set(1.0)
    gated = gate_weights * mask
    gate_weights = gated / (jnp.sum(gated, axis=-1, keepdims=True) + 1e-9)
    return np.asarray(gate_weights, dtype=np.float32)


def _partitions(items):
    """All set partitions of items (Bell number; fine for <= 8 experts)."""
    if not items:
        yield []
        return
    first, rest = items[0], items[1:]
    for part in _partitions(rest):
        for i in range(len(part)):
            yield part[:i] + [[first] + part[i]] + part[i + 1:]
        yield [[first]] + part


def _plan(gw):
    """Routing plan.  Each expert's routed rows are cut into k_e pieces
    placed on k_e distinct cores; experts are grouped into S slot classes
    (sum of k_e per class = N_CORES) so every core runs the same program
    structure (S slots with uniform capacities) while the slot->expert
    binding varies per core.  The grouping is chosen by exhaustive search
    minimizing PE stream + LDWEIGHTS + un-overlapped-DMA cost."""
    rows = {e: np.nonzero(gw[:, e])[0] for e in range(E)}
    active = [e for e in range(E) if len(rows[e])]
    counts = {e: len(rows[e]) for e in active}

    def class_caps(group):
        # piece counts proportional to size, each >=1, summing to N_CORES
        tot = sum(counts[e] for e in group)
        if len(group) > N_CORES:
            return None, None
        ks = {e: max(1, int(round(N_CORES * counts[e] / tot)))
              for e in group}
        while sum(ks.values()) > N_CORES:
            cands = [e for e in ks if ks[e] > 1]
            if not cands:
                return None, None
            big = max(cands, key=lambda e: ks[e] - N_CORES * counts[e] / tot)
            ks[big] -= 1
        while sum(ks.values()) < N_CORES:
            small = max(ks, key=lambda e: counts[e] / ks[e])
            ks[small] += 1
        cap = max(-(-counts[e] // ks[e]) for e in group)
        cap = (cap + 3) // 4 * 4
        return cap, ks

    STREAM_NS = 128 * (1 / 2.4)      # per column, both phases
    LDW_NS = 128 * 53.0              # per slot class (2*64 loads)
    best = None
    for part in _partitions(active):
        S = len(part)
        caps, kss = [], []
        ok = True
        for group in part:
            cap, ks = class_caps(group)
            if cap is None:
                ok = False
                break
            caps.append(cap)
            kss.append(ks)
        if not ok:
            continue
        stream = sum(caps) * STREAM_NS
        ldw = S * LDW_NS
        dma_ns = (S * 4.19e6 + 17.7e6) / 358e9 * 1e9
        cost = stream + ldw + max(0.0, dma_ns - 0.82 * stream)
        if best is None or cost < best[0]:
            best = (cost, part, caps, kss)

    _, part, caps, kss = best
    _cache["plan_info"] = {"counts": counts, "part": part, "caps": caps,
                           "kss": kss}
    # order classes big-first so the longest compute leads the pipeline
    order = np.argsort([-c for c in caps], kind="stable")
    part = [part[i] for i in order]
    caps = [caps[i] for i in order]
    kss = [kss[i] for i in order]

    # assign pieces to cores: class j occupies slot j on every core; experts
    # in a class take disjoint core ranges
    slot_expert = [[None] * len(part) for _ in range(N_CORES)]
    col_rows = [[] for _ in range(N_CORES)]
    for j, (group, cap, ks) in enumerate(zip(part, caps, kss)):
        core = 0
        for e in group:
            k = ks[e]
            pieces = np.array_split(rows[e], k)
            for p in pieces:
                slot_expert[core][j] = e
                pad = np.full(cap - len(p), -1, dtype=np.int64)
                col_rows[core].append(np.concatenate([p, pad]))
                core += 1
        assert core == N_CORES, (core, group, ks)
    col_rows = [np.concatenate(c) for c in col_rows]
    return slot_expert, caps, col_rows


def _chunks(cap):
    """Split cap into equal-width chunks <= CHUNK (multiples of 4, cap is),
    so no chunk is so narrow that LDWEIGHTS dominates the matmul."""
    nch = -(-cap // CHUNK)
    base = cap // nch // 4 * 4
    sizes = [base] * nch
    sizes[-1] = cap - base * (nch - 1)
    out, o = [], 0
    for s in sizes:
        out.append((o, s))
        o += s
    return out


def _build_program(repeats=1, structure=None):
    """Per-core Tile program for the routed-pairs expert MLP.
    `structure` = (n_slots, caps tuple); defaults to the structure of the
    last _make_in_maps call."""
    import concourse.bass as bass
    import concourse.mybir as mybir
    import concourse.tile as tile
    from concourse import bacc
    from contextlib import ExitStack

    if structure is None:
        structure = _cache["structure"]
    S, caps = structure
    ncol = sum(caps)

    f32 = mybir.dt.float32
    bf16 = mybir.dt.bfloat16
    AF = mybir.ActivationFunctionType

    nc = bacc.Bacc("TRN2", target_bir_lowering=False, debug=False,
                   num_devices=N_CORES)

    # xgt: per-slot contiguous blocks of Xg^T k-tiles: [128, sum_s KC*C_s]
    xgt = nc.dram_tensor("xgt", [128, KC * ncol], bf16, kind="ExternalInput").ap()
    w1 = nc.dram_tensor("w1", [S, 128, KC, P], bf16, kind="ExternalInput").ap()
    w2 = nc.dram_tensor("w2", [S, 128, PC, D], bf16, kind="ExternalInput").ap()
    gwc = nc.dram_tensor("gwc", [128, ncol], bf16, kind="ExternalInput").ap()
    b1t = nc.dram_tensor("b1t", [128, S * PC], f32, kind="ExternalInput").ap()
    # output as a flat sequence of [128, n] blocks in (slot, dt, chunk)
    # order, so every output DMA is fully contiguous in HBM
    ygt = nc.dram_tensor("ygt", [DT_TILES * 128 * ncol], bf16,
                         kind="ExternalOutput").ap()

    slot_off = np.concatenate([[0], np.cumsum(caps)])[:-1]

    with tile.TileContext(nc) as tc, ExitStack() as ctx:
        const = ctx.enter_context(tc.tile_pool(name="const", bufs=1))
        w1p = ctx.enter_context(tc.tile_pool(name="w1p", bufs=2))
        w2p = ctx.enter_context(tc.tile_pool(name="w2p", bufs=2))
        tmpp = ctx.enter_context(tc.tile_pool(name="tmpp", bufs=4))
        stgp = ctx.enter_context(tc.tile_pool(name="stgp", bufs=4))
        psp = ctx.enter_context(tc.tile_pool(name="psp", bufs=8, space="PSUM"))

        gwc_sb = const.tile([128, ncol], bf16)
        nc.sync.dma_start(out=gwc_sb[:], in_=gwc[:])
        b1_sb = const.tile([128, S * PC], f32)
        nc.sync.dma_start(out=b1_sb[:], in_=b1t[:])
        # per-slot Xg^T and h tiles (separate tiles -> per-slot DMA deps)
        xg_sb = [const.tile([128, KC, caps[s]], bf16, name=f"xg{s}")
                 for s in range(S)]
        h_sb = [const.tile([128, PC, caps[s]], bf16, name=f"h{s}")
                for s in range(S)]

        for rep in range(repeats):
            # activations re-DMAed per repeat so repeat-slope timing sees the
            # same steady-state DMA traffic as a single execution
            for s in range(S):
                nc.sync.dma_start(
                    out=xg_sb[s][:],
                    in_=xgt[:, KC * slot_off[s]:KC * (slot_off[s] + caps[s])])
            def phase1(s):
                # h[s] = gw * relu(Xg^T W1[s] + b1[s]); kc loop shared
                # across chunks so each stationary W1 tile is reused.
                chs = _chunks(caps[s])
                w1t = w1p.tile([128, KC, P], bf16, tag="w1")
                nc.sync.dma_start(out=w1t[:], in_=w1[s])
                for pc in range(PC):
                    pss = [psp.tile([128, CHUNK], f32, tag="ps",
                                    name=f"ps1_{rep}_{s}_{pc}_{i}")
                           for i in range(len(chs))]
                    for kc in range(KC):
                        for i, (off, n) in enumerate(chs):
                            nc.tensor.matmul(
                                pss[i][:, :n],
                                lhsT=w1t[:, kc, pc * 128:(pc + 1) * 128],
                                rhs=xg_sb[s][:, kc, off:off + n],
                                start=(kc == 0),
                                stop=(kc == KC - 1),
                            )
                    for i, (off, n) in enumerate(chs):
                        tmp = tmpp.tile([128, CHUNK], bf16, tag="tmp")
                        col = s * PC + pc
                        nc.scalar.activation(tmp[:, :n], pss[i][:, :n],
                                             AF.Relu,
                                             bias=b1_sb[:, col:col + 1])
                        nc.vector.tensor_mul(
                            h_sb[s][:, pc, off:off + n], tmp[:, :n],
                            gwc_sb[:, slot_off[s] + off:slot_off[s] + off + n])

            def phase2(s):
                # Yg^T blocks for slot s: W2[s]^T @ h[s]
                chs = _chunks(caps[s])
                w2t = w2p.tile([128, PC, D], bf16, tag="w2")
                nc.sync.dma_start(out=w2t[:], in_=w2[s])
                base = DT_TILES * 128 * slot_off[s]
                for dt in range(DT_TILES):
                    pss = [psp.tile([128, CHUNK], f32, tag="ps",
                                    name=f"ps2_{rep}_{s}_{dt}_{i}")
                           for i in range(len(chs))]
                    for pc in range(PC):
                        for i, (off, n) in enumerate(chs):
                            nc.tensor.matmul(
                                pss[i][:, :n],
                                lhsT=w2t[:, pc, dt * 128:(dt + 1) * 128],
                                rhs=h_sb[s][:, pc, off:off + n],
                                start=(pc == 0),
                                stop=(pc == PC - 1),
                            )
                    for i, (off, n) in enumerate(chs):
                        stg = stgp.tile([128, CHUNK], bf16, tag="stg")
                        nc.vector.tensor_copy(stg[:, :n], pss[i][:, :n])
                        pos = base + 128 * (caps[s] * dt + off)
                        nc.sync.dma_start(out=ygt[pos:pos + 128 * n],
                                          in_=stg[:, :n])

            # software-pipelined emission: p1(s+1) sits between p1(s) and
            # p2(s) so the PE never waits on the ACT/DVE h-handoff
            phase1(0)
            for s in range(S):
                if s + 1 < S:
                    phase1(s + 1)
                phase2(s)

    nc.compile()
    return nc


_cache = {}


def _get_program(structure):
    key = ("nc", structure)
    if key not in _cache:
        _cache[key] = _build_program(structure=structure)
    return _cache[key]


def _make_in_maps(inputs):
    x = np.asarray(inputs["query_embedding"], dtype=np.float32)
    W1 = np.asarray(inputs["W1"], dtype=np.float32)
    W2 = np.asarray(inputs["W2"], dtype=np.float32)
    b1 = np.asarray(inputs["b1"], dtype=np.float32)

    gw = _gate_weights(x, inputs["Wg"], inputs["bg"], inputs["Wp"],
                       inputs["bp"], inputs["Wgg"], inputs["bgg"])
    slot_expert, caps, col_rows = _plan(gw)
    S = len(caps)
    structure = (S, tuple(caps))
    _cache["structure"] = structure

    # per-expert prepacked weight blocks (built once, referenced per core)
    w1e = {e: np.ascontiguousarray(
        W1[e].astype(BF16).reshape(KC, 128, P).transpose(1, 0, 2))
        for e in set(e for se in slot_expert for e in se)}
    w2e = {e: np.ascontiguousarray(
        W2[e].astype(BF16).reshape(PC, 128, D).transpose(1, 0, 2))
        for e in w1e}
    b1e = {e: np.ascontiguousarray(
        b1[e].astype(np.float32).reshape(PC, 128).T) for e in w1e}

    xbf = x.astype(BF16)
    slot_off = np.concatenate([[0], np.cumsum(caps)])[:-1]
    in_maps = []
    for c in range(N_CORES):
        rows = col_rows[c]
        xg = xbf[np.maximum(rows, 0)]              # [NCOL, D]
        parts = []
        for s in range(S):
            blk = xg[slot_off[s]:slot_off[s] + caps[s]]   # [C_s, D]
            parts.append(np.ascontiguousarray(
                blk.T.reshape(KC, 128, caps[s]).transpose(1, 0, 2))
                .reshape(128, KC * caps[s]))
        xgt = np.ascontiguousarray(np.concatenate(parts, axis=1))
        experts_c = slot_expert[c]
        ecol = np.repeat(experts_c, caps)
        gcol = np.where(rows >= 0,
                        gw[np.maximum(rows, 0), ecol],
                        np.float32(0)).astype(BF16)
        gwc = np.ascontiguousarray(np.broadcast_to(gcol[None, :],
                                                   (128, len(rows))))
        w1h = np.stack([w1e[e] for e in experts_c])
        w2h = np.stack([w2e[e] for e in experts_c])
        b1h = np.concatenate([b1e[e] for e in experts_c], axis=1)
        in_maps.append({"xgt": xgt, "w1": w1h, "w2": w2h, "gwc": gwc,
                        "b1t": b1h})
    return in_maps, (gw, col_rows)


def _combine(ygt_list, col_rows, caps, gw, b2):
    """Sum each batch row's two expert partials (host gather-add).
    ygt_list[c]: flat block sequence (see _build_program's ygt layout)."""
    ncol = len(col_rows[0])
    slot_off = np.concatenate([[0], np.cumsum(caps)])[:-1]
    rows_all = np.concatenate(col_rows)
    y_all = np.empty((N_CORES * ncol, D), dtype=np.float32)
    for c in range(N_CORES):
        flat = np.asarray(ygt_list[c])
        yg = y_all[c * ncol:(c + 1) * ncol]
        for s, cap in enumerate(caps):
            base = DT_TILES * 128 * slot_off[s]
            for dt in range(DT_TILES):
                for off, n in _chunks(cap):
                    pos = base + 128 * (cap * dt + off)
                    blk = flat[pos:pos + 128 * n].reshape(128, n)
                    yg[slot_off[s] + off:slot_off[s] + off + n,
                       dt * 128:(dt + 1) * 128] = blk.T
    valid = rows_all >= 0
    order = np.argsort(rows_all[valid], kind="stable")
    ys = y_all[valid][order].reshape(B, 2, D)
    out = ys.sum(axis=1, dtype=np.float32)
    if np.any(b2):
        out = out + gw @ b2
    return out


def _run(inputs, trace=False):
    from concourse.bass_utils import run_bass_kernel_spmd

    in_maps, (gw, col_rows) = _make_in_maps(inputs)
    b2 = np.asarray(inputs["b2"], dtype=np.float32)

    nc = _get_program(_cache["structure"])
    res = run_bass_kernel_spmd(nc, in_maps, list(range(N_CORES)), trace=trace)
    out = _combine([res.results[c]["ygt"] for c in range(N_CORES)],
                   col_rows, list(_cache["structure"][1]), gw, b2)
    return out, res


def kernel(**inputs) -> np.ndarray:
    out, _ = _run(inputs, trace=False)
    return out


# revision 3
# speedup vs baseline: 1.2871x; 1.2871x over previous
"""Trainium2 Bass kernel for nn_EnhancedSpikingRetrievalCore (sparse MoE).

The reference computes all E=8 dense expert MLPs and then combines with
renormalized top-2 gate weights -- so 6/8 of the expert FLOPs multiply
into zero gate weights.  This kernel computes only the routed
(row, expert) pairs:

  host:   bit-exact gating (same jnp ops / platform as the reference)
          -> per-expert row lists -> deal each expert's rows round-robin
          across the 8 cores (per-core per-expert counts differ by <=1,
          so one SPMD program with uniform segment capacities serves all
          cores).  Columns of X are gathered host-side; the final
          combine (sum of each row's two expert partials) is a host
          sort-gather-add, i.e. pure unshard/gather work.
  device: per active expert e (capacity C_e columns per core):
            h_e   = gw_e * relu(Xg^T[e-cols] @ W1[e] + b1[e])   [P x C_e]
            Yg^T  = W2[e]^T @ h_e                               [D x C_e]
          Both matmuls keep batch in the moving (free) dimension so
          segment sizes can be exact; weights use native layouts as the
          stationary operand.

With ~2*B/8 + padding ~ 2060 columns per core instead of 8192, the PE
does ~4x less work than the dense kernel.  Expert partials are written
in bf16 (adds ~2e-4 rms on top of the ~2.7e-3 bf16 matmul error).
"""

import numpy as np
import ml_dtypes

B, D, E, G, P, H = 8192, 2048, 8, 4, 512, 192
T, DT_LIF, TAU, VTH, VRESET = 20, 0.001, 0.02, 0.5, 0.0
DELTA0, KTOP, KROUTE, PREDW = 7.0, 32, 2, 0.1

N_CORES = 8
KC = D // 128              # contraction chunks over D (16)
PC = P // 128              # chunks over P (4)
DT_TILES = D // 128        # output D row tiles (16)
CHUNK = 512                # moving-dim chunk (one PSUM bank of fp32)

BF16 = ml_dtypes.bfloat16


def _gate_weights(x, Wg, bg, Wp, bp, Wgg, bgg):
    """Renormalized top-2 gate weights [B, E], mirroring the reference
    gating ops verbatim (same jnp calls, default platform) so the
    ill-conditioned expert selection matches the oracle bit-for-bit."""
    import jax
    import jax.numpy as jnp

    x = jnp.asarray(x)
    dtype = x.dtype
    qm = jnp.mean(x, axis=-1)
    freqs = DELTA0 * jnp.arange(1, H + 1, dtype=dtype)
    ang = qm[:, None] * freqs[None, :]
    temporal = jnp.concatenate([jnp.cos(ang), jnp.sin(ang)], axis=-1)

    # mean over D of the spike rates is exactly KTOP/D for every row
    att_mean = jnp.full((x.shape[0],), np.float32(KTOP) / np.float32(D), dtype)
    gate_in = jnp.stack([jnp.mean(temporal, axis=-1), att_mean], axis=-1)

    gate_logits = gate_in @ jnp.asarray(Wg) + jnp.asarray(bg)
    gate_logits = gate_logits - PREDW * (gate_in @ jnp.asarray(Wp) + jnp.asarray(bp))
    group_logits = gate_in @ jnp.asarray(Wgg) + jnp.asarray(bgg)
    gmap = jax.nn.one_hot(jnp.arange(E) % G, G, dtype=dtype)
    gate_logits = gate_logits + group_logits @ gmap.T

    gate_weights = jax.nn.softmax(gate_logits, axis=-1)
    _, tidx = jax.lax.top_k(gate_weights, KROUTE)
    rows = jnp.arange(x.shape[0])[:, None]
    mask = jnp.zeros_like(gate_weights).at[rows, tidx].set(1.0)
    gated = gate_weights * mask
    gate_weights = gated / (jnp.sum(gated, axis=-1, keepdims=True) + 1e-9)
    return np.asarray(gate_weights, dtype=np.float32)


def _partitions(items):
    """All set partitions of items (Bell number; fine for <= 8 experts)."""
    if not items:
        yield []
        return
    first, rest = items[0], items[1:]
    for part in _partitions(rest):
        for i in range(len(part)):
            yield part[:i] + [[first] + part[i]] + part[i + 1:]
        yield [[first]] + part


def _plan(gw):
    """Routing plan.  Each expert's routed rows are cut into k_e pieces
    placed on k_e distinct cores; experts are grouped into S slot classes
    (sum of k_e per class = N_CORES) so every core runs the same program
    structure (S slots with uniform capacities) while the slot->expert
    binding varies per core.  The grouping is chosen by exhaustive search
    minimizing PE stream + LDWEIGHTS + un-overlapped-DMA cost."""
    rows = {e: np.nonzero(gw[:, e])[0] for e in range(E)}
    active = [e for e in range(E) if len(rows[e])]
    counts = {e: len(rows[e]) for e in active}

    def class_caps(group):
        # piece counts proportional to size, each >=1, summing to N_CORES
        tot = sum(counts[e] for e in group)
        if len(group) > N_CORES:
            return None, None
        ks = {e: max(1, int(round(N_CORES * counts[e] / tot)))
              for e in group}
        while sum(ks.values()) > N_CORES:
            cands = [e for e in ks if ks[e] > 1]
            if not cands:
                return None, None
            big = max(cands, key=lambda e: ks[e] - N_CORES * counts[e] / tot)
            ks[big] -= 1
        while sum(ks.values()) < N_CORES:
            small = max(ks, key=lambda e: counts[e] / ks[e])
            ks[small] += 1
        cap = max(-(-counts[e] // ks[e]) for e in group)
        cap = (cap + 3) // 4 * 4
        return cap, ks

    STREAM_NS = 128 * (1 / 2.4)      # per column, both phases
    LDW_NS = 128 * 53.0              # per slot class (2*64 loads)
    best = None
    for part in _partitions(active):
        S = len(part)
        caps, kss = [], []
        ok = True
        for group in part:
            cap, ks = class_caps(group)
            if cap is None:
                ok = False
                break
            caps.append(cap)
            kss.append(ks)
        if not ok:
            continue
        stream = sum(caps) * STREAM_NS
        ldw = S * LDW_NS
        dma_ns = (S * 4.19e6 + 17.7e6) / 358e9 * 1e9
        cost = stream + ldw + max(0.0, dma_ns - 0.82 * stream)
        if best is None or cost < best[0]:
            best = (cost, part, caps, kss)

    _, part, caps, kss = best
    _cache["plan_info"] = {"counts": counts, "part": part, "caps": caps,
                           "kss": kss}
    # order classes big-first so the longest compute leads the pipeline
    order = np.argsort([-c for c in caps], kind="stable")
    part = [part[i] for i in order]
    caps = [caps[i] for i in order]
    kss = [kss[i] for i in order]

    # assign pieces to cores: class j occupies slot j on every core; experts
    # in a class take disjoint core ranges
    slot_expert = [[None] * len(part) for _ in range(N_CORES)]
    col_rows = [[] for _ in range(N_CORES)]
    for j, (group, cap, ks) in enumerate(zip(part, caps, kss)):
        core = 0
        for e in group:
            k = ks[e]
            pieces = np.array_split(rows[e], k)
            for p in pieces:
                slot_expert[core][j] = e
                pad = np.full(cap - len(p), -1, dtype=np.int64)
                col_rows[core].append(np.concatenate([p, pad]))
                core += 1
        assert core == N_CORES, (core, group, ks)
    col_rows = [np.concatenate(c) for c in col_rows]
    return slot_expert, caps, col_rows


def _chunks(cap):
    """Split cap into equal-width chunks <= CHUNK (multiples of 4, cap is),
    so no chunk is so narrow that LDWEIGHTS dominates the matmul."""
    nch = -(-cap // CHUNK)
    base = cap // nch // 4 * 4
    sizes = [base] * nch
    sizes[-1] = cap - base * (nch - 1)
    out, o = [], 0
    for s in sizes:
        out.append((o, s))
        o += s
    return out


def _build_program(repeats=1, structure=None):
    """Per-core Tile program for the routed-pairs expert MLP.
    `structure` = (n_slots, caps tuple); defaults to the structure of the
    last _make_in_maps call."""
    import concourse.bass as bass
    import concourse.mybir as mybir
    import concourse.tile as tile
    from concourse import bacc
    from contextlib import ExitStack

    if structure is None:
        structure = _cache["structure"]
    S, caps = structure
    ncol = sum(caps)

    f32 = mybir.dt.float32
    bf16 = mybir.dt.bfloat16
    AF = mybir.ActivationFunctionType

    nc = bacc.Bacc("TRN2", target_bir_lowering=False, debug=False,
                   num_devices=N_CORES)

    # xgt: per-slot contiguous blocks of Xg^T k-tiles: [128, sum_s KC*C_s]
    xgt = nc.dram_tensor("xgt", [128, KC * ncol], bf16, kind="ExternalInput").ap()
    w1 = nc.dram_tensor("w1", [S, 128, KC, P], bf16, kind="ExternalInput").ap()
    w2 = nc.dram_tensor("w2", [S, 128, PC, D], bf16, kind="ExternalInput").ap()
    gwc = nc.dram_tensor("gwc", [128, ncol], bf16, kind="ExternalInput").ap()
    b1t = nc.dram_tensor("b1t", [128, S * PC], f32, kind="ExternalInput").ap()
    # output as a flat sequence of [128, n] blocks in (slot, dt, chunk)
    # order, so every output DMA is fully contiguous in HBM
    ygt = nc.dram_tensor("ygt", [DT_TILES * 128 * ncol], bf16,
                         kind="ExternalOutput").ap()

    slot_off = np.concatenate([[0], np.cumsum(caps)])[:-1]

    with tile.TileContext(nc) as tc, ExitStack() as ctx:
        const = ctx.enter_context(tc.tile_pool(name="const", bufs=1))
        w1p = ctx.enter_context(tc.tile_pool(name="w1p", bufs=2))
        w2p = ctx.enter_context(tc.tile_pool(name="w2p", bufs=2))
        tmpp = ctx.enter_context(tc.tile_pool(name="tmpp", bufs=4))
        stgp = ctx.enter_context(tc.tile_pool(name="stgp", bufs=4))
        psp = ctx.enter_context(tc.tile_pool(name="psp", bufs=8, space="PSUM"))

        gwc_sb = const.tile([128, ncol], bf16)
        nc.sync.dma_start(out=gwc_sb[:], in_=gwc[:])
        b1_sb = const.tile([128, S * PC], f32)
        nc.sync.dma_start(out=b1_sb[:], in_=b1t[:])
        # per-slot Xg^T and h tiles (separate tiles -> per-slot DMA deps)
        xg_sb = [const.tile([128, KC, caps[s]], bf16, name=f"xg{s}")
                 for s in range(S)]
        h_sb = [const.tile([128, PC, caps[s]], bf16, name=f"h{s}")
                for s in range(S)]

        for rep in range(repeats):
            # activations re-DMAed per repeat so repeat-slope timing sees the
            # same steady-state DMA traffic as a single execution; split per
            # k-tile so phase 1 can start as soon as the first tiles land
            for s in range(S):
                for kc in range(KC):
                    o = KC * slot_off[s] + kc * caps[s]
                    nc.sync.dma_start(out=xg_sb[s][:, kc, :],
                                      in_=xgt[:, o:o + caps[s]])
            def phase1(s):
                # h[s] = gw * relu(Xg^T W1[s] + b1[s]); kc loop shared
                # across chunks so each stationary W1 tile is reused.
                chs = _chunks(caps[s])
                w1t = w1p.tile([128, KC, P], bf16, tag="w1")
                for kc in range(KC):
                    nc.sync.dma_start(out=w1t[:, kc, :], in_=w1[s, :, kc, :])
                for pc in range(PC):
                    pss = [psp.tile([128, CHUNK], f32, tag="ps",
                                    name=f"ps1_{rep}_{s}_{pc}_{i}")
                           for i in range(len(chs))]
                    for kc in range(KC):
                        for i, (off, n) in enumerate(chs):
                            nc.tensor.matmul(
                                pss[i][:, :n],
                                lhsT=w1t[:, kc, pc * 128:(pc + 1) * 128],
                                rhs=xg_sb[s][:, kc, off:off + n],
                                start=(kc == 0),
                                stop=(kc == KC - 1),
                            )
                    for i, (off, n) in enumerate(chs):
                        tmp = tmpp.tile([128, CHUNK], bf16, tag="tmp")
                        col = s * PC + pc
                        nc.scalar.activation(tmp[:, :n], pss[i][:, :n],
                                             AF.Relu,
                                             bias=b1_sb[:, col:col + 1])
                        nc.vector.tensor_mul(
                            h_sb[s][:, pc, off:off + n], tmp[:, :n],
                            gwc_sb[:, slot_off[s] + off:slot_off[s] + off + n])

            def phase2(s):
                # Yg^T blocks for slot s: W2[s]^T @ h[s]
                chs = _chunks(caps[s])
                w2t = w2p.tile([128, PC, D], bf16, tag="w2")
                nc.sync.dma_start(out=w2t[:], in_=w2[s])
                base = DT_TILES * 128 * slot_off[s]
                for dt in range(DT_TILES):
                    pss = [psp.tile([128, CHUNK], f32, tag="ps",
                                    name=f"ps2_{rep}_{s}_{dt}_{i}")
                           for i in range(len(chs))]
                    for pc in range(PC):
                        for i, (off, n) in enumerate(chs):
                            nc.tensor.matmul(
                                pss[i][:, :n],
                                lhsT=w2t[:, pc, dt * 128:(dt + 1) * 128],
                                rhs=h_sb[s][:, pc, off:off + n],
                                start=(pc == 0),
                                stop=(pc == PC - 1),
                            )
                    for i, (off, n) in enumerate(chs):
                        stg = stgp.tile([128, CHUNK], bf16, tag="stg")
                        nc.vector.tensor_copy(stg[:, :n], pss[i][:, :n])
                        pos = base + 128 * (caps[s] * dt + off)
                        nc.sync.dma_start(out=ygt[pos:pos + 128 * n],
                                          in_=stg[:, :n])

            # software-pipelined emission: p1(s+1) sits between p1(s) and
            # p2(s) so the PE never waits on the ACT/DVE h-handoff
            phase1(0)
            for s in range(S):
                if s + 1 < S:
                    phase1(s + 1)
                phase2(s)

    nc.compile()
    return nc


_cache = {}


def _get_program(structure):
    key = ("nc", structure)
    if key not in _cache:
        _cache[key] = _build_program(structure=structure)
    return _cache[key]


def _make_in_maps(inputs):
    x = np.asarray(inputs["query_embedding"], dtype=np.float32)
    W1 = np.asarray(inputs["W1"], dtype=np.float32)
    W2 = np.asarray(inputs["W2"], dtype=np.float32)
    b1 = np.asarray(inputs["b1"], dtype=np.float32)

    gw = _gate_weights(x, inputs["Wg"], inputs["bg"], inputs["Wp"],
                       inputs["bp"], inputs["Wgg"], inputs["bgg"])
    slot_expert, caps, col_rows = _plan(gw)
    S = len(caps)
    structure = (S, tuple(caps))
    _cache["structure"] = structure

    # per-expert prepacked weight blocks (built once, referenced per core)
    w1e = {e: np.ascontiguousarray(
        W1[e].astype(BF16).reshape(KC, 128, P).transpose(1, 0, 2))
        for e in set(e for se in slot_expert for e in se)}
    w2e = {e: np.ascontiguousarray(
        W2[e].astype(BF16).reshape(PC, 128, D).transpose(1, 0, 2))
        for e in w1e}
    b1e = {e: np.ascontiguousarray(
        b1[e].astype(np.float32).reshape(PC, 128).T) for e in w1e}

    xbf = x.astype(BF16)
    slot_off = np.concatenate([[0], np.cumsum(caps)])[:-1]
    in_maps = []
    for c in range(N_CORES):
        rows = col_rows[c]
        xg = xbf[np.maximum(rows, 0)]              # [NCOL, D]
        parts = []
        for s in range(S):
            blk = xg[slot_off[s]:slot_off[s] + caps[s]]   # [C_s, D]
            parts.append(np.ascontiguousarray(
                blk.T.reshape(KC, 128, caps[s]).transpose(1, 0, 2))
                .reshape(128, KC * caps[s]))
        xgt = np.ascontiguousarray(np.concatenate(parts, axis=1))
        experts_c = slot_expert[c]
        ecol = np.repeat(experts_c, caps)
        gcol = np.where(rows >= 0,
                        gw[np.maximum(rows, 0), ecol],
                        np.float32(0)).astype(BF16)
        gwc = np.ascontiguousarray(np.broadcast_to(gcol[None, :],
                                                   (128, len(rows))))
        w1h = np.stack([w1e[e] for e in experts_c])
        w2h = np.stack([w2e[e] for e in experts_c])
        b1h = np.concatenate([b1e[e] for e in experts_c], axis=1)
        in_maps.append({"xgt": xgt, "w1": w1h, "w2": w2h, "gwc": gwc,
                        "b1t": b1h})
    return in_maps, (gw, col_rows)


def _combine(ygt_list, col_rows, caps, gw, b2):
    """Sum each batch row's two expert partials (host gather-add).
    ygt_list[c]: flat block sequence (see _build_program's ygt layout)."""
    ncol = len(col_rows[0])
    slot_off = np.concatenate([[0], np.cumsum(caps)])[:-1]
    rows_all = np.concatenate(col_rows)
    y_all = np.empty((N_CORES * ncol, D), dtype=np.float32)
    for c in range(N_CORES):
        flat = np.asarray(ygt_list[c])
        yg = y_all[c * ncol:(c + 1) * ncol]
        for s, cap in enumerate(caps):
            base = DT_TILES * 128 * slot_off[s]
            for dt in range(DT_TILES):
                for off, n in _chunks(cap):
                    pos = base + 128 * (cap * dt + off)
                    blk = flat[pos:pos + 128 * n].reshape(128, n)
                    yg[slot_off[s] + off:slot_off[s] + off + n,
                       dt * 128:(dt + 1) * 128] = blk.T
    valid = rows_all >= 0
    order = np.argsort(rows_all[valid], kind="stable")
    ys = y_all[valid][order].reshape(B, 2, D)
    out = ys.sum(axis=1, dtype=np.float32)
    if np.any(b2):
        out = out + gw @ b2
    return out


def _run(inputs, trace=False):
    from concourse.bass_utils import run_bass_kernel_spmd

    in_maps, (gw, col_rows) = _make_in_maps(inputs)
    b2 = np.asarray(inputs["b2"], dtype=np.float32)

    nc = _get_program(_cache["structure"])
    res = run_bass_kernel_spmd(nc, in_maps, list(range(N_CORES)), trace=trace)
    out = _combine([res.results[c]["ygt"] for c in range(N_CORES)],
                   col_rows, list(_cache["structure"][1]), gw, b2)
    return out, res


def kernel(**inputs) -> np.ndarray:
    out, _ = _run(inputs, trace=False)
    return out
